# revision 43
# baseline (speedup 1.0000x reference)
"""FBSNN forward pass on 8 Trainium2 NeuronCores (Bass/Tile).

Problem: 2048 Monte-Carlo paths x 21 time points through a 4x1024 tanh MLP
(forward + gradient wrt X), GBM log-Euler path, squared-residual loss.

Key algebra: the log-Euler path telescopes,
    X_n = Xi * exp((mu - sigma^2/2) * (t_n - t_0) + (W_n - W_0) @ L^T)
and Y_path[n] = u(t_n, X_n), so every (path, time) sample is independent ->
one big batched MLP fwd+bwd over 43008 rows, data-parallel over paths on
8 cores (256 paths/core). Loss terms only couple consecutive time points of
the same path -> shifted elementwise ops with masks; per-core partial losses
summed on the host.

Device layout: activations stored transposed ([features on partitions, rows
on free dim]) so no inter-layer transposes are needed. Matmuls in bf16 with
fp32 PSUM accumulation; the X path, reductions and loss math in fp32.
The backward pass uses g = (h^2 - 1) * gpre per layer (one fused
scalar_tensor_tensor per chunk); the sign flip alternates per layer and
cancels over the 4 tanh layers, so the final dX equals +DuDx.
"""

import numpy as np
import ml_dtypes

# ---- problem constants (hardcoded per contract) ----
D = 100
M = 2048
NP1 = 21            # time points
WIDTH = 1024
MU, SIG, RHO = 0.06, 0.2, 0.5
RRATE, KSTRIKE = 0.05, 1.0
DRIFT = MU - 0.5 * SIG * SIG

NCORES = 8
MC = M // NCORES    # 256 paths per core
RT = MC * NP1       # 5376 rows per core
KP = 16             # paths per tile
R = KP * NP1        # 336 columns per tile
NT = MC // KP       # 16 tiles per core
FC = WIDTH // 128   # 8 feature chunks

_CACHE = {}
_PROFILE = False
_LAST_RESULTS = None


def _chol():
    sig = np.full((D,), SIG, np.float32)
    cov = (sig[:, None] * sig[None, :]) * (RHO + (1.0 - RHO) * np.eye(D, dtype=np.float32))
    return np.linalg.cholesky(cov).astype(np.float32)


def _build_program():
    import concourse.mybir as mybir
    import concourse.tile as tile
    from concourse import bacc
    from concourse.bass import ds
    from contextlib import ExitStack

    F = mybir.dt.float32
    BF = mybir.dt.bfloat16
    AF = mybir.ActivationFunctionType
    ALU = mybir.AluOpType

    # Bacc (not raw Bass): its compile() legalizes multi-wait instructions
    # (move_matmul_waits_to_ldweights + generate_event_semaphores) — raw Bass
    # programs with >1 sync wait per instruction are rejected by walrus.
    nc = bacc.Bacc()

    # ---- DRAM parameters ----
    xin = nc.declare_dram_parameter("xin", [D + 1, RT], F, False)     # rows 0-99 (W_n-W_0)^T, row 100 t_n-t_0
    dw2t = nc.declare_dram_parameter("dw2t", [D, RT], F, False)       # (W_n - W_{n-1})^T, 0 at n=0
    trowb = nc.declare_dram_parameter("trowb", [1, RT], BF, False)    # raw t, bf16 hi
    trowlo = nc.declare_dram_parameter("trowlo", [1, RT], BF, False)  # t - bf16(t), bf16 lo
    rows3 = nc.declare_dram_parameter("rows3", [3, RT], F, False)     # 1+R*dt, n0mask, termmask
    w0xd = nc.declare_dram_parameter("w0x", [D, WIDTH], BF, False)
    w0td = nc.declare_dram_parameter("w0t", [1, WIDTH], BF, False)
    wfd = [nc.declare_dram_parameter(f"w{i}", [WIDTH, WIDTH], BF, False) for i in (1, 2, 3)]
    wtd = [nc.declare_dram_parameter(f"w{i}t", [WIDTH, WIDTH], BF, False) for i in (1, 2, 3)]
    w0btd = nc.declare_dram_parameter("w0bt", [WIDTH, D], BF, False)
    w4d = nc.declare_dram_parameter("w4", [WIDTH], BF, False)
    w4fd = nc.declare_dram_parameter("w4f", [WIDTH], F, False)
    bd = [nc.declare_dram_parameter(f"b{i}", [WIDTH], F, False) for i in range(4)]
    b4d = nc.declare_dram_parameter("b4", [1, 1], F, False)
    laugd = nc.declare_dram_parameter("laug", [D + 1, D], F, False)   # [L^T ; drift row]
    xid = nc.declare_dram_parameter("xi", [D, 1], F, False)

    xout = nc.declare_dram_parameter("xout", [RT, D], F, True)
    yout = nc.declare_dram_parameter("yout", [1, RT], F, True)
    lout = nc.declare_dram_parameter("lout", [RT, 2], F, True)
    touchout = nc.declare_dram_parameter("touchout", [1, 32], F, True)

    identd = nc.inline_tensor(np.eye(D, dtype=np.float32), name="ident")

    with ExitStack() as ctx:
        tc = ctx.enter_context(tile.TileContext(nc))
        singles = ctx.enter_context(tc.tile_pool(name="singles", bufs=1))
        work = ctx.enter_context(tc.tile_pool(name="work", bufs=2))
        hpool = ctx.enter_context(tc.tile_pool(name="hpool", bufs=1))
        gpool = ctx.enter_context(tc.tile_pool(name="gpool", bufs=2))
        psbig = ctx.enter_context(tc.tile_pool(name="psbig", bufs=2, space="PSUM"))
        psxw = ctx.enter_context(tc.tile_pool(name="psxw", bufs=3, space="PSUM"))
        psdx = ctx.enter_context(tc.tile_pool(name="psdx", bufs=1, space="PSUM"))
        psvec = ctx.enter_context(tc.tile_pool(name="psvec", bufs=2, space="PSUM"))

        # ---- load constants / weights into SBUF ----
        w0xs = singles.tile([D, WIDTH], BF, tag="w0xs")
        nc.sync.dma_start(out=w0xs, in_=w0xd[:])
        w0ts = singles.tile([1, WIDTH], BF, tag="w0ts")
        nc.sync.dma_start(out=w0ts, in_=w0td[:])
        wfs, wts = [], []
        for i in range(3):
            wf = singles.tile([128, FC, WIDTH], BF, tag=f"wf{i}")
            nc.sync.dma_start(out=wf, in_=wfd[i][:].rearrange("(kc p) n -> p kc n", p=128))
            wfs.append(wf)
            wt = singles.tile([128, FC, WIDTH], BF, tag=f"wt{i}")
            nc.sync.dma_start(out=wt, in_=wtd[i][:].rearrange("(kc p) n -> p kc n", p=128))
            wts.append(wt)
        w0bts = singles.tile([128, FC, D], BF, tag="w0bts")
        nc.sync.dma_start(out=w0bts, in_=w0btd[:].rearrange("(kc p) n -> p kc n", p=128))
        w4s = singles.tile([128, FC], BF, tag="w4s")
        nc.sync.dma_start(out=w4s, in_=w4d[:].rearrange("(kc p) -> p kc", p=128))
        w4fs = singles.tile([128, FC], F, tag="w4fs")
        nc.sync.dma_start(out=w4fs, in_=w4fd[:].rearrange("(kc p) -> p kc", p=128))
        bs = []
        for i in range(4):
            b = singles.tile([128, FC], F, tag=f"bs{i}")
            nc.sync.dma_start(out=b, in_=bd[i][:].rearrange("(kc p) -> p kc", p=128))
            bs.append(b)
        b4s = singles.tile([1, 1], F, tag="b4s")
        nc.sync.dma_start(out=b4s, in_=b4d[:])
        laugs = singles.tile([D + 1, D], F, tag="laugs")
        nc.sync.dma_start(out=laugs, in_=laugd[:])
        xis = singles.tile([D, 1], F, tag="xis")
        nc.sync.dma_start(out=xis, in_=xid[:])
        idents = singles.tile([D, D], F, tag="idents")
        nc.sync.dma_start(out=idents, in_=identd[:])
        oness = singles.tile([D, 1], F, tag="oness")
        nc.vector.memset(oness, 1.0)
        kbias = singles.tile([1, 1], F, tag="kbias")
        nc.vector.memset(kbias, -float(KSTRIKE))

        # ---- preamble "touch" ops ----
        # The TPB ISA has one wait slot per instruction; walrus rejects
        # matmuls that need >1 semaphore wait. Touch every DMA'd operand once
        # on the engine that will consume it, so the engine's vector clock
        # already covers that DMA lane when the real consumers issue.
        warm = psbig.tile([128, R], F, tag="big")
        pe_touch = [laugs, w0xs, w0ts, wfs[0], wfs[1], wfs[2],
                    wts[0], wts[1], wts[2], w0bts, w4s, oness, idents]
        for j, tt in enumerate(pe_touch):
            p = tt.shape[0]
            col = tt[0:p, 0, 0:1] if len(tt.shape) == 3 else tt[0:p, 0:1]
            nc.tensor.matmul(warm[0:1, j:j + 1], col, col, start=True, stop=True)
        scr_dve = work.tile([1, 8], F, tag="scr_dve")
        for j, tt in enumerate([w4fs, b4s, xis]):
            nc.vector.tensor_copy(scr_dve[0:1, j:j + 1], tt[0:1, 0:1])
        scr_act = work.tile([1, 8], F, tag="scr_act")
        for j, tt in enumerate([bs[0], bs[1], bs[2], bs[3], kbias]):
            nc.scalar.copy(scr_act[0:1, j:j + 1], tt[0:1, 0:1])
        scr_pe = work.tile([1, 16], F, tag="scr_pe")
        nc.vector.tensor_copy(scr_pe[0:1, 0:len(pe_touch)], warm[0:1, 0:len(pe_touch)])
        # anchor the touch chain to a real output so DCE keeps it
        nc.sync.dma_start(out=touchout[0:1, 0:8], in_=scr_dve)
        nc.sync.dma_start(out=touchout[0:1, 8:16], in_=scr_act)
        nc.sync.dma_start(out=touchout[0:1, 16:32], in_=scr_pe)

        for _it in range(NT):
            iv = _it * R
            # ---- load this tile's inputs ----
            xin_sb = work.tile([D + 1, R], F, tag="xin_sb")
            nc.sync.dma_start(out=xin_sb, in_=xin[:, ds(iv, R)])
            dw2_sb = work.tile([D, R], F, tag="dw2_sb")
            nc.sync.dma_start(out=dw2_sb, in_=dw2t[:, ds(iv, R)])
            onep_sb = work.tile([1, R], F, tag="onep_sb")
            nc.sync.dma_start(out=onep_sb, in_=rows3[0:1, ds(iv, R)])
            n0m_sb = work.tile([1, R], F, tag="n0m_sb")
            nc.sync.dma_start(out=n0m_sb, in_=rows3[1:2, ds(iv, R)])
            term_sb = work.tile([1, R], F, tag="term_sb")
            nc.sync.dma_start(out=term_sb, in_=rows3[2:3, ds(iv, R)])
            trow_sb = work.tile([1, R], BF, tag="trow_sb")
            nc.sync.dma_start(out=trow_sb, in_=trowb[0:1, ds(iv, R)])
            trowlo_sb = work.tile([1, R], BF, tag="trowlo_sb")
            nc.sync.dma_start(out=trowlo_sb, in_=trowlo[0:1, ds(iv, R)])

            # ---- X path: exp(L@dW^T + drift*t') * Xi, all fp32 ----
            xps = psxw.tile([D, R], F, tag="xw")
            nc.tensor.matmul(xps, laugs[:, :], xin_sb[:, :], start=True, stop=True)
            ldps = psxw.tile([D, R], F, tag="xw")
            nc.tensor.matmul(ldps, laugs[0:D, :], dw2_sb[:, :], start=True, stop=True)
            ldw_sb = work.tile([D, R], F, tag="ldw_sb")
            nc.vector.tensor_copy(ldw_sb, ldps)
            xexp = work.tile([D, R], F, tag="xexp")
            nc.scalar.activation(xexp, xps, AF.Exp)
            xfin = work.tile([D, R], F, tag="xfin")
            nc.vector.tensor_scalar_mul(xfin, xexp, xis[:, 0:1])
            # activation-split: every forward operand is a (hi, lo) bf16 pair
            # so activation rounding cancels; only weight rounding remains.
            ztx = work.tile([D, R], BF, tag="ztx")
            nc.vector.tensor_copy(ztx, xfin)   # bf16 hi
            ztxlo = work.tile([D, R], BF, tag="ztxlo")
            nc.vector.tensor_sub(ztxlo, xfin, ztx)   # bf16 lo residual

            # ---- MLP forward (transposed activations, bf16 hi+lo pairs) ----
            hhis = []
            prev_hi, prev_lo = None, None
            for l in range(4):
                hhi = hpool.tile([128, FC, R], BF, tag=f"hhi{l}", name=f"hhi{l}")
                hlo = gpool.tile([128, FC, R], BF, tag="hlo", name=f"hlo{l}")
                for fc in range(FC):
                    ps = psbig.tile([128, R], F, tag="big")
                    fsl = slice(fc * 128, (fc + 1) * 128)
                    if l == 0:
                        nc.tensor.matmul(ps, w0xs[:, fsl], ztx[:, :], start=True, stop=False)
                        nc.tensor.matmul(ps, w0xs[:, fsl], ztxlo[:, :], start=False, stop=False)
                        nc.tensor.matmul(ps, w0ts[:, fsl], trow_sb[:, :], start=False, stop=False)
                        nc.tensor.matmul(ps, w0ts[:, fsl], trowlo_sb[:, :], start=False, stop=True)
                    else:
                        for kc in range(FC):
                            nc.tensor.matmul(ps, wfs[l - 1][:, kc, fsl],
                                             prev_hi[:, kc, :], start=(kc == 0), stop=False)
                            nc.tensor.matmul(ps, wfs[l - 1][:, kc, fsl],
                                             prev_lo[:, kc, :], start=False, stop=(kc == FC - 1))
                    h32 = work.tile([128, R], F, tag="h32")
                    nc.scalar.activation(h32, ps, AF.Tanh, bias=bs[l][:, fc:fc + 1])
                    nc.vector.tensor_copy(hhi[:, fc, :], h32)
                    nc.vector.tensor_sub(hlo[:, fc, :], h32, hhi[:, fc, :])
                hhis.append(hhi)
                prev_hi, prev_lo = hhi, hlo

            ups = psvec.tile([1, R], F, tag="vec")
            for kc in range(FC):
                nc.tensor.matmul(ups, w4s[:, kc:kc + 1], prev_hi[:, kc, :],
                                 start=(kc == 0), stop=False)
                nc.tensor.matmul(ups, w4s[:, kc:kc + 1], prev_lo[:, kc, :],
                                 start=False, stop=(kc == FC - 1))
            u_sb = work.tile([1, R], F, tag="u_sb")
            nc.vector.tensor_scalar_add(u_sb, ups, b4s[0:1, 0:1])

            # ---- backward (negated gradients): g = (h^2 - 1) * gpre ----
            g4 = gpool.tile([128, FC, R], BF, tag="g")
            for fc in range(FC):
                hh = work.tile([128, R], F, tag="hh")
                nc.scalar.activation(hh, hhis[3][:, fc, :], AF.Square)
                nc.vector.tensor_scalar(g4[:, fc, :], hh, 1.0, w4fs[:, fc:fc + 1],
                                        ALU.subtract, ALU.mult)
            gprev = g4
            for l in (2, 1, 0):
                g = gpool.tile([128, FC, R], BF, tag="g")
                for kc in range(FC):
                    ps = psbig.tile([128, R], F, tag="big")
                    for fc in range(FC):
                        nc.tensor.matmul(ps, wts[l][:, fc, kc * 128:(kc + 1) * 128],
                                         gprev[:, fc, :], start=(fc == 0), stop=(fc == FC - 1))
                    hh = work.tile([128, R], F, tag="hh")
                    nc.scalar.activation(hh, hhis[l][:, kc, :], AF.Square)
                    nc.vector.scalar_tensor_tensor(g[:, kc, :], hh, 1.0, ps,
                                                   ALU.subtract, ALU.mult)
                gprev = g
            dxps = psdx.tile([D, R], F, tag="dx")
            for kc in range(FC):
                nc.tensor.matmul(dxps, w0bts[:, kc, :], gprev[:, kc, :],
                                 start=(kc == 0), stop=(kc == FC - 1))

            # ---- loss pieces ----
            v_sb = work.tile([D, R], F, tag="v_sb")
            nc.vector.memset(v_sb[:, 0:1], 0.0)
            nc.vector.tensor_mul(v_sb[:, 1:R], dxps[:, 0:R - 1], ldw_sb[:, 1:R])
            dps = psvec.tile([1, R], F, tag="vec")
            nc.tensor.matmul(dps, oness[:, :], v_sb[:, :], start=True, stop=True)
            sxps = psvec.tile([1, R], F, tag="vec")
            nc.tensor.matmul(sxps, oness[:, :], xfin[:, :], start=True, stop=True)

            # sign bookkeeping: g = (h^2-1)*gpre flips sign EVERY layer, so after
            # the 4 tanh layers dxps = +DuDx (the flips cancel pairwise) and the
            # dot term enters tilde with a PLUS sign: c = 1 + R*dt + dot.
            c_sb = work.tile([1, R], F, tag="c_sb")
            nc.vector.tensor_add(c_sb, onep_sb, dps)
            tilde = work.tile([1, R], F, tag="tilde")
            nc.vector.memset(tilde[:, 0:1], 0.0)
            nc.vector.tensor_mul(tilde[0:1, 1:R], u_sb[0:1, 0:R - 1], c_sb[0:1, 1:R])
            res = work.tile([1, R], F, tag="res")
            nc.vector.tensor_sub(res, u_sb, tilde)
            resm = work.tile([1, R], F, tag="resm")
            nc.vector.tensor_mul(resm, res, n0m_sb)
            losspair = work.tile([1, 2], F, tag="losspair")
            sq = work.tile([1, R], F, tag="sq")
            nc.scalar.activation(sq, resm, AF.Square, accum_out=losspair[0:1, 0:1])

            gterm = work.tile([1, R], F, tag="gterm")
            nc.scalar.activation(gterm, sxps, AF.Relu, bias=kbias[0:1, 0:1], scale=1.0 / D)
            rt_ = work.tile([1, R], F, tag="rt_")
            nc.vector.tensor_sub(rt_, u_sb, gterm)
            rtm = work.tile([1, R], F, tag="rtm")
            nc.vector.tensor_mul(rtm, rt_, term_sb)
            sq2 = work.tile([1, R], F, tag="sq2")
            nc.scalar.activation(sq2, rtm, AF.Square, accum_out=losspair[0:1, 1:2])

            # ---- outputs ----
            nc.sync.dma_start(out=yout[0:1, ds(iv, R)], in_=u_sb)
            nc.sync.dma_start(out=lout[ds(iv, 1), :], in_=losspair)
            for c3 in range(3):
                tps = psdx.tile([112, D], F, tag="dx")
                nc.tensor.transpose(tps, xfin[:, c3 * 112:(c3 + 1) * 112], idents)
                xstage = work.tile([112, D], F, tag="xstage")
                nc.vector.tensor_copy(xstage, tps)
                nc.sync.dma_start(out=xout[ds(iv + c3 * 112, 112), :], in_=xstage)

    nc.compile()
    return nc


def _prep_inputs(t, W, Xi, params):
    bf = ml_dtypes.bfloat16
    L = _chol()
    (W0, b0), (W1, b1), (W2, b2), (W3, b3), (W4, b4) = [
        (np.ascontiguousarray(np.asarray(w)), np.ascontiguousarray(np.asarray(b))) for w, b in params
    ]
    shared = {
        "w0x": np.ascontiguousarray(W0[1:, :]).astype(bf),
        "w0t": np.ascontiguousarray(W0[0:1, :]).astype(bf),
        "w1": W1.astype(bf), "w2": W2.astype(bf), "w3": W3.astype(bf),
        "w1t": np.ascontiguousarray(W1.T).astype(bf),
        "w2t": np.ascontiguousarray(W2.T).astype(bf),
        "w3t": np.ascontiguousarray(W3.T).astype(bf),
        "w0bt": np.ascontiguousarray(W0[1:, :].T).astype(bf),
        "w4": W4.reshape(WIDTH).astype(bf),
        "w4f": W4.reshape(WIDTH).astype(np.float32),
        "b0": b0.astype(np.float32), "b1": b1.astype(np.float32),
        "b2": b2.astype(np.float32), "b3": b3.astype(np.float32),
        "b4": b4.reshape(1, 1).astype(np.float32),
        "laug": np.vstack([L.T, np.full((1, D), DRIFT, np.float32)]).astype(np.float32),
        "xi": np.asarray(Xi, np.float32).reshape(D, 1),
    }
    n0mask = np.tile((np.arange(NP1) != 0).astype(np.float32), MC)
    termmask = np.tile((np.arange(NP1) == NP1 - 1).astype(np.float32), MC)
    t = np.asarray(t, np.float32)
    W = np.asarray(W, np.float32)
    in_maps = []
    for c in range(NCORES):
        tcs = t[c * MC:(c + 1) * MC]
        Wcs = W[c * MC:(c + 1) * MC]
        tp = (tcs - tcs[:, :1, :]).reshape(RT)
        dWfull = (Wcs - Wcs[:, :1, :]).reshape(RT, D)
        xin = np.empty((D + 1, RT), np.float32)
        xin[:D] = dWfull.T
        xin[D] = tp
        dW2 = np.zeros_like(Wcs)
        dW2[:, 1:, :] = Wcs[:, 1:, :] - Wcs[:, :-1, :]
        dt = np.zeros((MC, NP1), np.float32)
        dt[:, 1:] = tcs[:, 1:, 0] - tcs[:, :-1, 0]
        m = dict(shared)
        m.update({
            "xin": xin,
            "dw2t": np.ascontiguousarray(dW2.reshape(RT, D).T),
            "trowb": tcs.reshape(1, RT).astype(bf),
        "trowlo": (tcs.reshape(1, RT) - tcs.reshape(1, RT).astype(bf).astype(np.float32)).astype(bf),
            "rows3": np.ascontiguousarray(
                np.stack([(1.0 + RRATE * dt).reshape(RT), n0mask, termmask]).astype(np.float32)),
        })
        in_maps.append(m)
    return in_maps


def kernel(t, W, Xi, params):
    global _LAST_RESULTS
    from concourse.bass_utils import run_bass_kernel_spmd

    if "nc" not in _CACHE:
        _CACHE["nc"] = _build_program()
    nc = _CACHE["nc"]

    in_maps = _prep_inputs(t, W, Xi, params)
    kw = {}
    if _PROFILE:
        kw = dict(trace=True)
    res = run_bass_kernel_spmd(nc, in_maps, list(range(NCORES)), **kw)
    _LAST_RESULTS = res

    X_path = np.empty((M, NP1, D), np.float32)
    Y_path = np.empty((M, NP1, 1), np.float32)
    loss = np.float64(0.0)
    for c in range(NCORES):
        r = res.results[c]
        X_path[c * MC:(c + 1) * MC] = np.asarray(r["xout"], np.float32).reshape(MC, NP1, D)
        Y_path[c * MC:(c + 1) * MC] = np.asarray(r["yout"], np.float32).reshape(MC, NP1, 1)
        loss += np.asarray(r["lout"], np.float64)[np.arange(NT) * R, :].sum()
    return (np.float32(loss), X_path, Y_path, np.float32(Y_path[0, 0, 0]))
